# revision 6
# baseline (speedup 1.0000x reference)
"""Affinity-propagation spatial stencil kernel for Trainium2 (8 NeuronCores).

Data-parallel: 16 images sharded 2-per-core; the 2 images of a core are
MERGED into the free dimension of every tile ([P, 2, RPP, W]), halving
instruction count and per-op overhead.

Math (A_k = zero-padded shift by OFFSETS[k]; G_k = guidance channel k):
  absw = sum_k A_k |G_k|;  inv = 1/absw = exp(-ln(absw))
  gate_sum = (sum_k A_k G_k) * inv;  bias = raw - gate_sum * raw
  step:  r' = inv * (sum_k A_k (G_k * r)) + bias
The shift of a product collapses: (A_k G_k) * (A_k r) = A_k (G_k * r), so
products are unshifted elementwise muls; only the shift-SUM needs moves.
Shifted sums group by row-shift class c in {-1,0,+1}:
  sum_k A_k x_k = rowshift_+(u_p) + u_0 + rowshift_-(u_m)
Column shifts are free-dim AP offsets (tiles carry zero guard columns).
Row shifts: partition p holds 4 image rows, so 3 of 4 rows shift within
the partition (free-dim row offset); the partition-crossing row is
computed on the idle TensorEngine as a matmul with a sub/super-diagonal
0/1 stationary matrix into PSUM (halo_dn[m] = up[m+1, row0] etc).  This
replaces the baseline's per-step halo DMAs, which serialized ~157us on a
single DMA engine (queue 10).

Engine split per step (times ~us, merged 4096-elem ops):
  DVE : 7 products (2.3 ea), up/u0 trees, combine, x inv, + bias  (~31)
  GP  : 1 product + the 2 um-tree adds (tensor ops are 3.6x slower
        on GPSIMD, eff 0.42 -> give it exactly the off-critical slice)
  ACT : all fp32->fp16 casts of the loads, ln/exp (setup only)
  PE  : 4 shift matmuls per combine (halo rows)
abs runs as tensor_scalar(abs_max, 0) on DVE -- 4x_2p mode (0.25x).
"""

import sys

sys.path.insert(0, "/opt/trn_rl_repo")

import numpy as np

import concourse.bass as bass
import concourse.mybir as mybir
from concourse import tile
from concourse.bass_utils import run_bass_kernel_spmd

N_CORES = 8
B, K, H, W = 16, 8, 512, 512
BPC = B // N_CORES  # images per core (merged in free dim)
P = 128
RPP = H // P  # rows per partition
WG = W + 4  # guarded row width (image cols at [2:514])
C0 = 2
PROP_TIME = 4

F32 = mybir.dt.float32
I32 = mybir.dt.int32
DT = mybir.dt.float16
AT = mybir.AluOpType
AF = mybir.ActivationFunctionType


def _split_excess_waits(nc):
    """This walrus build encodes at most 1 sem wait per instruction; move the
    overflow onto preceding NoOps. Also drop EVENT_SEMAPHORE_RANGE_CLEAR
    (unencodable here; only appears at the kernel tail where it's a no-op)."""
    for f in nc.m.functions:
        for bb in f.blocks:
            new_insts = []
            for ins in bb.instructions:
                if getattr(ins, "op_name", None) == "EVENT_SEMAPHORE_RANGE_CLEAR":
                    continue
                cap = 1
                si = getattr(ins, "sync_info", None)
                if si is not None and si.on_wait and len(si.on_wait) > cap:
                    extra = list(si.on_wait[cap:])
                    del si.on_wait[cap:]
                    while extra:
                        nop = mybir.InstNoOp(
                            name=nc.get_next_instruction_name(),
                            engine=ins.engine,
                            sync_info=mybir.SyncInfo(on_wait=extra[:cap], on_update=[]),
                        )
                        new_insts.append(nop)
                        extra = extra[cap:]
                new_insts.append(ins)
            bb.instructions[:] = new_insts


def _c(ap):
    """center (image) view of a guarded [P, BPC, RPP, WG] tile."""
    return ap[:, :, :, C0 : C0 + W]


def _w(ap, dj):
    """column-shifted view of a guarded tile: value at [i, j+dj]."""
    return ap[:, :, :, C0 + dj : C0 + dj + W]


def _emit(nc, pool, psum, g_dram, d_dram, o_dram):
    V = nc.vector
    GP = nc.gpsimd
    ACT = nc.scalar
    PE = nc.tensor

    def gtile(name):  # guarded work tile
        return pool.tile([P, BPC, RPP, WG], DT, name=name)

    def utile(name):  # unguarded work tile
        return pool.tile([P, BPC, RPP, W], DT, name=name)

    gates = pool.tile([P, BPC, K, RPP, WG], DT, name="gates")
    stage = pool.tile([P, RPP, W], F32, name="stage", bufs=2)
    sA, sB, sC, sD, sE = (gtile(n) for n in ("sA", "sB", "sC", "sD", "sE"))
    upA, u0A, umA = utile("upA"), utile("u0A"), utile("umA")
    upB, u0B, umB = utile("upB"), utile("u0B"), utile("umB")
    rA = gtile("rA")
    inv = utile("inv")
    bias = utile("bias")
    # shift matrices for the PE halo: halo_dn[m]=x[m+1], halo_up[m]=x[m-1]
    wdn = pool.tile([P, P], DT, name="wdn")
    wup = pool.tile([P, P], DT, name="wup")
    ci = pool.tile([P, P], F32, name="ci")
    pm1 = pool.tile([P, 1], F32, name="pm1")
    pp1 = pool.tile([P, 1], F32, name="pp1")
    psum_dn = psum.tile([P, BPC, W], F32, name="psum_dn", bufs=2)
    psum_up = psum.tile([P, BPC, W], F32, name="psum_up", bufs=2)

    def gv(k):  # guarded gate plane view [P, BPC, RPP, WG]
        return gates[:, :, k]

    def gw(k, dj):  # column-shifted gate view
        return gates[:, :, k, :, C0 + dj : C0 + dj + W]

    # ---- constants: shift matrices via iota + is_equal ----
    GP.iota(ci[:], [[1, P]], base=0, channel_multiplier=0,
            allow_small_or_imprecise_dtypes=True)  # ci[p,j] = j
    GP.iota(pm1[:], [[1, 1]], base=-1, channel_multiplier=1,
            allow_small_or_imprecise_dtypes=True)  # p-1
    GP.iota(pp1[:], [[1, 1]], base=1, channel_multiplier=1,
            allow_small_or_imprecise_dtypes=True)  # p+1
    # wdn[p,m] = 1 iff p == m+1  <=>  m == p-1 ; wup[p,m]=1 iff m == p+1
    V.tensor_scalar(wdn[:], ci[:], pm1[:, 0:1], None, AT.is_equal)
    V.tensor_scalar(wup[:], ci[:], pp1[:, 0:1], None, AT.is_equal)

    # ---- zero guard columns (written once; ops below write centers only) ----
    for b in range(BPC):
        GP.memset(gates[:, b, :, :, 0:C0], 0.0)
        GP.memset(gates[:, b, :, :, C0 + W : WG], 0.0)
    for t in (sA, sB, sC, sD, sE, rA):
        GP.memset(t[:, :, :, 0:C0], 0.0)
        GP.memset(t[:, :, :, C0 + W : WG], 0.0)

    # ---- loads (nc.sync -> queue striped over all 16 DMA engines) ----
    # depth first: raw is needed by bias and step 1
    for b in range(BPC):
        nc.sync.dma_start(
            out=stage[:], in_=d_dram[b, 0].rearrange("(p r) j -> p r j", p=P)
        )
        ACT.activation(rA[:, b, :, C0 : C0 + W], stage[:], AF.Copy)

    # gate loads, streaming: per k convert (ACT for k<5, DVE for k>=5 to
    # balance the two engines inside the DMA window), abs on ACT (merged
    # over both images), and feed both the absw tree (upA/u0A/umA over |g|)
    # and the gate-sum tree (upB/u0B/umB over g).
    def _abs(dst, k):
        ACT.activation(_c(dst), _c(gv(k)), AF.Abs)

    for k in range(K):
        for b in range(BPC):
            nc.sync.dma_start(
                out=stage[:], in_=g_dram[b, k].rearrange("(p r) j -> p r j", p=P)
            )
            if k < 5:
                ACT.activation(gates[:, b, k, :, C0 : C0 + W], stage[:], AF.Copy)
            else:
                V.tensor_copy(gates[:, b, k, :, C0 : C0 + W], stage[:])
        if k == 0:
            _abs(sA, 0)
        elif k == 1:
            _abs(sB, 1)
            V.tensor_add(upA[:], _w(sA, 1), _c(sB))
            V.tensor_add(upB[:], gw(0, 1), gw(1, 0))
        elif k == 2:
            _abs(sA, 2)
            V.tensor_add(upA[:], upA[:], _w(sA, -1))
            V.tensor_add(upB[:], upB[:], gw(2, -1))
        elif k == 3:
            _abs(sB, 3)
        elif k == 4:
            _abs(sC, 4)
            V.tensor_add(u0A[:], _w(sB, 1), _w(sC, -1))
            V.tensor_add(u0B[:], gw(3, 1), gw(4, -1))
        elif k == 5:
            _abs(sD, 5)
        elif k == 6:
            _abs(sE, 6)
            GP.tensor_add(umA[:], _w(sD, 1), _c(sE))
            GP.tensor_add(umB[:], gw(5, 1), gw(6, 0))
        elif k == 7:
            _abs(sA, 7)
            GP.tensor_add(umA[:], umA[:], _w(sA, -1))
            GP.tensor_add(umB[:], umB[:], gw(7, -1))

    def halo(up_t, um_t):
        """PE: psum_dn[m] = up[m+1, row0]; psum_up[m] = um[m-1, row3]."""
        for b in range(BPC):
            PE.matmul(
                out=psum_dn[:, b, :], lhsT=wdn[:], rhs=up_t[:, b, 0, :],
                start=True, stop=True,
            )
            PE.matmul(
                out=psum_up[:, b, :], lhsT=wup[:], rhs=um_t[:, b, 3, :],
                start=True, stop=True,
            )

    def combine(dst, up_t, u0_t, um_t, edges_on_gp=False):
        """dst[q] = up[q+1] + u0[q] + um[q-1] with PE-halo partition rows."""
        halo(up_t, um_t)
        E = GP if edges_on_gp else V
        V.tensor_add(dst[:, :, 0:3, :], up_t[:, :, 1:4, :], u0_t[:, :, 0:3, :])
        E.tensor_add(dst[:, :, 3, :], psum_dn[:], u0_t[:, :, 3, :])
        V.tensor_add(dst[:, :, 1:4, :], dst[:, :, 1:4, :], um_t[:, :, 0:3, :])
        E.tensor_add(dst[:, :, 0, :], dst[:, :, 0, :], psum_up[:])

    # ---- absw -> inv = exp(-ln(absw)) ; combine lands in `bias` scratch ----
    combine(bias, upA, u0A, umA)
    for b in range(BPC):
        ACT.activation(stage[:], bias[:, b], AF.Ln)
        ACT.activation(inv[:, b], stage[:], AF.Exp, scale=-1.0)

    # ---- gate_sum -> bias = raw - gate_sum*raw ; combine lands in u0A ----
    combine(u0A, upB, u0B, umB)
    V.tensor_mul(u0A[:], u0A[:], inv[:])
    V.tensor_mul(u0A[:], u0A[:], _c(rA))
    V.tensor_sub(bias[:], _c(rA), u0A[:])

    # ---- propagation, r updated in place in rA ----
    for step in range(PROP_TIME):
        # GP owns g7's product and the um tree; DVE feeds it g5,g6 first.
        V.tensor_mul(_c(sA), _c(gv(5)), _c(rA))
        V.tensor_mul(_c(sB), _c(gv(6)), _c(rA))
        GP.tensor_mul(_c(sE), _c(gv(7)), _c(rA))
        V.tensor_mul(_c(sC), _c(gv(0)), _c(rA))
        V.tensor_mul(_c(sD), _c(gv(1)), _c(rA))
        GP.tensor_add(umA[:], _w(sA, 1), _c(sB))
        V.tensor_add(upA[:], _w(sC, 1), _c(sD))
        V.tensor_mul(_c(sC), _c(gv(2)), _c(rA))
        V.tensor_add(upA[:], upA[:], _w(sC, -1))
        V.tensor_mul(_c(sD), _c(gv(3)), _c(rA))
        V.tensor_mul(_c(sC), _c(gv(4)), _c(rA))
        GP.tensor_add(umA[:], umA[:], _w(sE, -1))
        V.tensor_add(u0A[:], _w(sD, 1), _w(sC, -1))
        combine(upB, upA, u0A, umA)
        if step < PROP_TIME - 1:
            V.tensor_mul(upB[:], upB[:], inv[:])
            V.tensor_add(_c(rA), upB[:], bias[:])
        else:
            for b in range(BPC):
                V.tensor_mul(stage[:], upB[:, b], inv[:, b])
                V.tensor_add(stage[:], stage[:], bias[:, b])
                nc.sync.dma_start(
                    out=o_dram[b, 0].rearrange("(p r) j -> p r j", p=P),
                    in_=stage[:],
                )


def build(legalize=True):
    nc = bass.Bass()
    g_dram = nc.declare_dram_parameter("guidance", [BPC, K, H, W], F32, isOutput=False)
    d_dram = nc.declare_dram_parameter("blur_depth", [BPC, 1, H, W], F32, isOutput=False)
    o_dram = nc.declare_dram_parameter("out", [BPC, 1, H, W], F32, isOutput=True)
    with tile.TileContext(nc) as tc:
        with tc.tile_pool(name="main", bufs=1) as pool:
            with tc.tile_pool(name="ps", space="PSUM", bufs=1) as psum:
                _emit(nc, pool, psum, g_dram, d_dram, o_dram)
    if legalize:
        _split_excess_waits(nc)
    return nc


_NC = None


def _get_nc():
    global _NC
    if _NC is None:
        _NC = build()
    return _NC


def run(guidance, blur_depth, **spmd_kwargs):
    nc = _get_nc()
    in_maps = [
        {
            "guidance": np.ascontiguousarray(guidance[BPC * c : BPC * (c + 1)]),
            "blur_depth": np.ascontiguousarray(blur_depth[BPC * c : BPC * (c + 1)]),
        }
        for c in range(N_CORES)
    ]
    res = run_bass_kernel_spmd(nc, in_maps, list(range(N_CORES)), **spmd_kwargs)
    out = np.concatenate([res.results[i]["out"] for i in range(N_CORES)], axis=0)
    return out, res


def kernel(guidance, blur_depth):
    out, _ = run(guidance, blur_depth)
    return out.astype(np.float32)


# revision 14
# speedup vs baseline: 1.1505x; 1.1505x over previous
"""Affinity-propagation spatial stencil kernel for Trainium2 (8 NeuronCores).

Data-parallel: 16 images sharded 2-per-core; a core's 2 images are merged
into the free dimension as 8 flattened rows-per-partition ([P, 8, W]:
rows 4b..4b+3 belong to image b), so every engine op uses a 2-level
free access pattern (the DVE 2x fp16 mode and the GPSIMD ucode both
degrade on deeper APs).

Math (A_k = zero-padded shift by OFFSETS[k]; G_k = guidance channel k):
  absw = sum_k A_k |G_k|;  inv = 1/absw = exp(-ln(absw))
  gate_sum = (sum_k A_k G_k) * inv;  bias = raw - gate_sum * raw
  step:  r' = inv * (sum_k A_k (G_k * r)) + bias
(A_k G_k)*(A_k r) = A_k (G_k * r): products are unshifted muls; only the
shift-SUM moves data.  Column shifts ride free-dim AP offsets (guard
columns); row shifts act within a partition's 4 rows except the
partition-crossing row, which the idle TensorEngine produces as a matmul
with a sub/super-diagonal 0/1 stationary into PSUM (halo_dn[m] =
up[m+1, row0]).  ACT drains PSUM to SBUF fp16 (DVE reading PSUM directly
measured ~10x slow); the edge adds then run on fp16 in SBUF.

Per-step engine split: DVE products g0..g6 + up/u0 trees + combine +
inv/bias; GPSIMD the g7 product + um tree + one combine row-add; ACT the
2 PSUM drains; PE 4 halo matmuls.  Setup streams loads through 3
rotating stage buffers (depth-3 pipeline ~ HBM bound), converts on
ACT (k<4) / DVE (k>=4), abs on ACT, and runs the absw and gate-sum trees
behind the loads.
"""

import sys

sys.path.insert(0, "/opt/trn_rl_repo")

import numpy as np

import concourse.bass as bass
import concourse.mybir as mybir
from concourse import tile
from concourse.bass_utils import run_bass_kernel_spmd

N_CORES = 8
B, K, H, W = 16, 8, 512, 512
BPC = B // N_CORES  # images per core (merged: 8 rows per partition)
P = 128
RPP = H // P  # rows per partition per image
R2 = BPC * RPP  # flattened rows per partition
WG = W + 4  # guarded row width (image cols at [2:514])
C0 = 2
PROP_TIME = 4

F32 = mybir.dt.float32
DT = mybir.dt.float16
AT = mybir.AluOpType
AF = mybir.ActivationFunctionType


def _split_excess_waits(nc):
    """This walrus build encodes at most 1 sem wait per instruction; move the
    overflow onto preceding NoOps. Also drop EVENT_SEMAPHORE_RANGE_CLEAR
    (unencodable here; only appears at the kernel tail where it's a no-op)."""
    for f in nc.m.functions:
        for bb in f.blocks:
            new_insts = []
            for ins in bb.instructions:
                if getattr(ins, "op_name", None) == "EVENT_SEMAPHORE_RANGE_CLEAR":
                    continue
                cap = 1
                si = getattr(ins, "sync_info", None)
                if si is not None and si.on_wait and len(si.on_wait) > cap:
                    extra = list(si.on_wait[cap:])
                    del si.on_wait[cap:]
                    while extra:
                        nop = mybir.InstNoOp(
                            name=nc.get_next_instruction_name(),
                            engine=ins.engine,
                            sync_info=mybir.SyncInfo(on_wait=extra[:cap], on_update=[]),
                        )
                        new_insts.append(nop)
                        extra = extra[cap:]
                new_insts.append(ins)
            bb.instructions[:] = new_insts


def _c(ap):
    """center (image) view of a guarded [P, R2, WG] tile."""
    return ap[:, :, C0 : C0 + W]


def _w(ap, dj):
    """column-shifted view of a guarded tile: value at [i, j+dj]."""
    return ap[:, :, C0 + dj : C0 + dj + W]


def _emit(nc, pool, psum, g_dram, d_dram, o_dram):
    V = nc.vector
    GP = nc.gpsimd
    ACT = nc.scalar
    PE = nc.tensor

    def gtile(name):  # guarded work tile
        return pool.tile([P, R2, WG], DT, name=name)

    def utile(name):  # unguarded work tile
        return pool.tile([P, R2, W], DT, name=name)

    gates = pool.tile([P, K, R2, WG], DT, name="gates")
    stages = [pool.tile([P, RPP, W], F32, name=f"stg{i}") for i in range(3)]
    # sA..sD: DVE product scratches (prop) / abs ping-pong (setup);
    # sE: gpsimd's g7 product scratch.
    sA, sB, sC, sD, sE = (gtile(n) for n in ("sA", "sB", "sC", "sD", "sE"))
    # PSUM halo drain target: fp16 view of the third stage buffer (free
    # outside the load window): rows 0,1 = halo_dn b0,b1; rows 2,3 = halo_up.
    halo = stages[2].bitcast(DT)
    upA, u0A, umA = utile("upA"), utile("u0A"), utile("umA")
    upB, u0B, umB = utile("upB"), utile("u0B"), utile("umB")
    rA = gtile("rA")
    inv = utile("inv")
    bias = utile("bias")
    # shift matrices for the PE halo: halo_dn[m]=x[m+1], halo_up[m]=x[m-1]
    wdn = pool.tile([P, P], DT, name="wdn")
    wup = pool.tile([P, P], DT, name="wup")
    ci = pool.tile([P, P], F32, name="ci")
    pm1 = pool.tile([P, 1], F32, name="pm1")
    pp1 = pool.tile([P, 1], F32, name="pp1")
    psum_dn = psum.tile([P, BPC, W], F32, name="psum_dn", bufs=2)
    psum_up = psum.tile([P, BPC, W], F32, name="psum_up", bufs=2)

    def gv(k, dj=0):  # column-shifted gate view [P, R2, W]
        return gates[:, k, :, C0 + dj : C0 + dj + W]

    # ---- constants: shift matrices via iota + is_equal ----
    GP.iota(ci[:], [[1, P]], base=0, channel_multiplier=0,
            allow_small_or_imprecise_dtypes=True)  # ci[p,j] = j
    GP.iota(pm1[:], [[1, 1]], base=-1, channel_multiplier=1,
            allow_small_or_imprecise_dtypes=True)  # p-1
    GP.iota(pp1[:], [[1, 1]], base=1, channel_multiplier=1,
            allow_small_or_imprecise_dtypes=True)  # p+1
    # wdn[p,m] = 1 iff p == m+1  <=>  m == p-1 ; wup[p,m] = 1 iff m == p+1
    V.tensor_scalar(wdn[:], ci[:], pm1[:, 0:1], None, AT.is_equal)
    V.tensor_scalar(wup[:], ci[:], pp1[:, 0:1], None, AT.is_equal)

    # ---- zero guard columns (written once; ops below write centers only) ----
    GP.memset(gates[:, :, :, 0:C0], 0.0)
    GP.memset(gates[:, :, :, C0 + W : WG], 0.0)
    for t in (sA, sB, sC, sD, sE, rA):
        GP.memset(t[:, :, 0:C0], 0.0)
        GP.memset(t[:, :, C0 + W : WG], 0.0)

    # ---- loads: depth-3 pipeline over 3 rotating stage buffers ----
    load_i = 0

    def load(dram_plane, cast_eng, dst):
        nonlocal load_i
        st = stages[load_i % 3]
        load_i += 1
        nc.sync.dma_start(out=st[:], in_=dram_plane.rearrange("(p r) j -> p r j", p=P))
        if cast_eng is V:
            V.tensor_copy(dst, st[:])
        else:
            ACT.activation(dst, st[:], AF.Copy)

    for b in range(BPC):
        load(d_dram[b, 0], ACT, rA[:, 4 * b : 4 * b + 4, C0 : C0 + W])

    # abs ping-pong: even k -> sA, odd k -> sB (product scratches, free here)
    def _abs(k):
        dst = sA if k % 2 == 0 else sB
        ACT.activation(_c(dst), gv(k), AF.Abs)
        return dst

    for k in range(K):
        eng = ACT if k < 4 else V
        for b in range(BPC):
            load(g_dram[b, k], eng, gates[:, k, 4 * b : 4 * b + 4, C0 : C0 + W])
        s = _abs(k)
        if k == 1:
            V.tensor_add(upA[:], _w(sA, 1), _c(sB))
            V.tensor_add(upB[:], gv(0, 1), gv(1))
        elif k == 2:
            V.tensor_add(upA[:], upA[:], _w(sA, -1))
            V.tensor_add(upB[:], upB[:], gv(2, -1))
        elif k == 4:
            V.tensor_add(u0A[:], _w(sB, 1), _w(sA, -1))
            V.tensor_add(u0B[:], gv(3, 1), gv(4, -1))
        elif k == 6:
            GP.tensor_add(umA[:], _w(sB, 1), _c(sA))
            GP.tensor_add(umB[:], gv(5, 1), gv(6))
        elif k == 7:
            GP.tensor_add(umA[:], umA[:], _w(sB, -1))
            GP.tensor_add(umB[:], umB[:], gv(7, -1))

    HDN = halo[:, 0:BPC, 0:W]  # drained halo_dn rows (b = 0, 1)
    HUP = halo[:, BPC : 2 * BPC, 0:W]  # drained halo_up rows

    def combine(dst, up_t, u0_t, um_t, um_add_gp=False):
        """dst[q] = up[q+1] + u0[q] + um[q-1] per image (q in 0..3)."""
        for b in range(BPC):
            PE.matmul(out=psum_dn[:, b, :], lhsT=wdn[:], rhs=up_t[:, 4 * b, :],
                      start=True, stop=True)
        for b in range(BPC):
            PE.matmul(out=psum_up[:, b, :], lhsT=wup[:], rhs=um_t[:, 4 * b + 3, :],
                      start=True, stop=True)
        ACT.activation(HDN, psum_dn[:], AF.Copy)
        ACT.activation(HUP, psum_up[:], AF.Copy)
        V.tensor_add(dst[:, 3::RPP, :], HDN, u0_t[:, 3::RPP, :])
        for b in range(BPC):
            V.tensor_add(dst[:, 4 * b : 4 * b + 3, :],
                         up_t[:, 4 * b + 1 : 4 * b + 4, :],
                         u0_t[:, 4 * b : 4 * b + 3, :])
        for b in range(BPC):
            E = GP if (um_add_gp and b == 0) else V
            E.tensor_add(dst[:, 4 * b + 1 : 4 * b + 4, :],
                         dst[:, 4 * b + 1 : 4 * b + 4, :],
                         um_t[:, 4 * b : 4 * b + 3, :])
        V.tensor_add(dst[:, 0::RPP, :], dst[:, 0::RPP, :], HUP)

    # ---- absw -> inv = exp(-ln(absw)); combine lands in `bias` scratch ----
    combine(bias, upA, u0A, umA)
    for b in range(BPC):
        ACT.activation(stages[b][:], bias[:, 4 * b : 4 * b + 4, :], AF.Ln)
        ACT.activation(inv[:, 4 * b : 4 * b + 4, :], stages[b][:], AF.Exp, scale=-1.0)

    # ---- gate_sum -> bias = raw - gate_sum*raw; combine lands in u0A ----
    combine(u0A, upB, u0B, umB)
    V.tensor_mul(u0A[:], u0A[:], inv[:])
    V.tensor_mul(u0A[:], u0A[:], _c(rA))
    V.tensor_sub(bias[:], _c(rA), u0A[:])

    # ---- propagation, r updated in place in rA ----
    for step in range(PROP_TIME):
        V.tensor_mul(_c(sA), gv(5), _c(rA))
        V.tensor_mul(_c(sB), gv(6), _c(rA))
        GP.tensor_mul(_c(sE), gv(7), _c(rA))
        V.tensor_mul(_c(sC), gv(0), _c(rA))
        V.tensor_mul(_c(sD), gv(1), _c(rA))
        GP.tensor_add(umA[:], _w(sA, 1), _c(sB))
        V.tensor_add(upA[:], _w(sC, 1), _c(sD))
        V.tensor_mul(_c(sC), gv(2), _c(rA))
        V.tensor_add(upA[:], upA[:], _w(sC, -1))
        V.tensor_mul(_c(sD), gv(3), _c(rA))
        V.tensor_mul(_c(sC), gv(4), _c(rA))
        GP.tensor_add(umA[:], umA[:], _w(sE, -1))
        V.tensor_add(u0A[:], _w(sD, 1), _w(sC, -1))
        combine(upB, upA, u0A, umA, um_add_gp=True)
        if step < PROP_TIME - 1:
            V.tensor_mul(upB[:], upB[:], inv[:])
            V.tensor_add(_c(rA), upB[:], bias[:])
        else:
            for b in range(BPC):
                st = stages[b]
                V.tensor_mul(st[:], upB[:, 4 * b : 4 * b + 4, :],
                             inv[:, 4 * b : 4 * b + 4, :])
                V.tensor_add(st[:], st[:], bias[:, 4 * b : 4 * b + 4, :])
                nc.sync.dma_start(
                    out=o_dram[b, 0].rearrange("(p r) j -> p r j", p=P), in_=st[:]
                )


def build(legalize=True):
    nc = bass.Bass()
    g_dram = nc.declare_dram_parameter("guidance", [BPC, K, H, W], F32, isOutput=False)
    d_dram = nc.declare_dram_parameter("blur_depth", [BPC, 1, H, W], F32, isOutput=False)
    o_dram = nc.declare_dram_parameter("out", [BPC, 1, H, W], F32, isOutput=True)
    with tile.TileContext(nc) as tc:
        with tc.tile_pool(name="main", bufs=1) as pool:
            with tc.tile_pool(name="ps", space="PSUM", bufs=1) as psum:
                _emit(nc, pool, psum, g_dram, d_dram, o_dram)
    if legalize:
        _split_excess_waits(nc)
    return nc


_NC = None


def _get_nc():
    global _NC
    if _NC is None:
        _NC = build()
    return _NC


def run(guidance, blur_depth, **spmd_kwargs):
    nc = _get_nc()
    in_maps = [
        {
            "guidance": np.ascontiguousarray(guidance[BPC * c : BPC * (c + 1)]),
            "blur_depth": np.ascontiguousarray(blur_depth[BPC * c : BPC * (c + 1)]),
        }
        for c in range(N_CORES)
    ]
    res = run_bass_kernel_spmd(nc, in_maps, list(range(N_CORES)), **spmd_kwargs)
    out = np.concatenate([res.results[i]["out"] for i in range(N_CORES)], axis=0)
    return out, res


def kernel(guidance, blur_depth):
    out, _ = run(guidance, blur_depth)
    return out.astype(np.float32)


# revision 21
# speedup vs baseline: 1.8165x; 1.5790x over previous
"""Affinity-propagation spatial stencil kernel for Trainium2 (8 NeuronCores).

Data-parallel: 16 images sharded 2-per-core; a core's 2 images are merged
into the free dimension as 8 flattened rows-per-partition ([P, 8, W]:
rows 4b..4b+3 belong to image b), so every engine op uses a 2-level
free access pattern (the DVE 2x fp16 mode and the GPSIMD ucode both
degrade on deeper APs).

Math (A_k = zero-padded shift by OFFSETS[k]; G_k = guidance channel k):
  absw = sum_k A_k |G_k|;  inv = 1/absw = exp(-ln(absw))
  gate_sum = (sum_k A_k G_k) * inv;  bias = raw - gate_sum * raw
  step:  r' = inv * (sum_k A_k (G_k * r)) + bias
(A_k G_k)*(A_k r) = A_k (G_k * r): products are unshifted muls; only the
shift-SUM moves data.  Column shifts ride free-dim AP offsets (guard
columns); row shifts act within a partition's 4 rows except the
partition-crossing row, which the idle TensorEngine produces as a matmul
with a sub/super-diagonal 0/1 stationary into PSUM (halo_dn[m] =
up[m+1, row0]).  ACT drains PSUM to SBUF fp16 (DVE reading PSUM directly
measured ~10x slow); the edge adds then run on fp16 in SBUF.

Per-step engine split: DVE products g0..g6 + up/u0 trees + combine +
inv/bias; GPSIMD the g7 product + um tree + one combine row-add; ACT the
2 PSUM drains; PE 4 halo matmuls.  Setup streams loads through 3
rotating stage buffers (depth-3 pipeline ~ HBM bound), converts on
ACT (k<4) / DVE (k>=4), abs on ACT, and runs the absw and gate-sum trees
behind the loads.
"""

import sys

sys.path.insert(0, "/opt/trn_rl_repo")

import numpy as np

import concourse.bass as bass
import concourse.mybir as mybir
from concourse import tile
from concourse.bass_utils import run_bass_kernel_spmd

N_CORES = 8
B, K, H, W = 16, 8, 512, 512
BPC = B // N_CORES  # images per core (merged: 8 rows per partition)
P = 128
RPP = H // P  # rows per partition per image
R2 = BPC * RPP  # flattened rows per partition
WG = W + 4  # guarded row width (image cols at [2:514])
C0 = 2
PROP_TIME = 4

F32 = mybir.dt.float32
DT = mybir.dt.float16
AT = mybir.AluOpType
AF = mybir.ActivationFunctionType


def _split_excess_waits(nc):
    """This walrus build encodes at most 1 sem wait per instruction; move the
    overflow onto preceding NoOps. Also drop EVENT_SEMAPHORE_RANGE_CLEAR
    (unencodable here; only appears at the kernel tail where it's a no-op)."""
    for f in nc.m.functions:
        for bb in f.blocks:
            new_insts = []
            for ins in bb.instructions:
                if getattr(ins, "op_name", None) == "EVENT_SEMAPHORE_RANGE_CLEAR":
                    continue
                cap = 1
                si = getattr(ins, "sync_info", None)
                if si is not None and si.on_wait and len(si.on_wait) > cap:
                    extra = list(si.on_wait[cap:])
                    del si.on_wait[cap:]
                    while extra:
                        nop = mybir.InstNoOp(
                            name=nc.get_next_instruction_name(),
                            engine=ins.engine,
                            sync_info=mybir.SyncInfo(on_wait=extra[:cap], on_update=[]),
                        )
                        new_insts.append(nop)
                        extra = extra[cap:]
                new_insts.append(ins)
            bb.instructions[:] = new_insts


def _c(ap):
    """center (image) view of a guarded [P, R2, WG] tile."""
    return ap[:, :, C0 : C0 + W]


def _w(ap, dj):
    """column-shifted view of a guarded tile: value at [i, j+dj]."""
    return ap[:, :, C0 + dj : C0 + dj + W]


def _emit(nc, pool, psum, g_dram, d_dram, o_dram):
    V = nc.vector
    GP = nc.gpsimd
    ACT = nc.scalar
    PE = nc.tensor

    def gtile(name):  # guarded work tile
        return pool.tile([P, R2, WG], DT, name=name)

    def utile(name):  # unguarded work tile
        return pool.tile([P, R2, W], DT, name=name)

    gates = pool.tile([P, K, R2, WG], DT, name="gates")
    stages = [pool.tile([P, RPP, W], F32, name=f"stg{i}") for i in range(4)]
    # sA..sC: DVE product scratches (prop); sA/sB double as the abs
    # ping-pong in setup.
    sA, sB, sC = (gtile(n) for n in ("sA", "sB", "sC"))
    upA, u0A, umA = utile("upA"), utile("u0A"), utile("umA")
    upB, u0B, umB = utile("upB"), utile("u0B"), utile("umB")
    rA = gtile("rA")
    inv = utile("inv")
    bias = utile("bias")
    # shift matrices for the PE halo: halo_dn[m]=x[m+1], halo_up[m]=x[m-1];
    # wid = identity (accumulates in-partition rows into the same PSUM bank)
    wdn = pool.tile([P, P], DT, name="wdn")
    wup = pool.tile([P, P], DT, name="wup")
    wid = pool.tile([P, P], DT, name="wid")
    ci = pool.tile([P, P], F32, name="ci")
    pm1 = pool.tile([P, 1], F32, name="pm1")
    pp1 = pool.tile([P, 1], F32, name="pp1")
    pz0 = pool.tile([P, 1], F32, name="pz0")
    psum_dn = psum.tile([P, BPC, W], F32, name="psum_dn", bufs=2)
    psum_up = psum.tile([P, BPC, W], F32, name="psum_up", bufs=2)

    def gv(k, dj=0):  # column-shifted gate view [P, R2, W]
        return gates[:, k, :, C0 + dj : C0 + dj + W]

    # ---- constants: shift matrices via iota + is_equal ----
    GP.iota(ci[:], [[1, P]], base=0, channel_multiplier=0,
            allow_small_or_imprecise_dtypes=True)  # ci[p,j] = j
    GP.iota(pm1[:], [[1, 1]], base=-1, channel_multiplier=1,
            allow_small_or_imprecise_dtypes=True)  # p-1
    GP.iota(pp1[:], [[1, 1]], base=1, channel_multiplier=1,
            allow_small_or_imprecise_dtypes=True)  # p+1
    GP.iota(pz0[:], [[1, 1]], base=0, channel_multiplier=1,
            allow_small_or_imprecise_dtypes=True)  # p
    # wdn[p,m] = 1 iff p == m+1  <=>  m == p-1 ; wup[p,m] = 1 iff m == p+1
    V.tensor_scalar(wdn[:], ci[:], pm1[:, 0:1], None, AT.is_equal)
    V.tensor_scalar(wup[:], ci[:], pp1[:, 0:1], None, AT.is_equal)
    V.tensor_scalar(wid[:], ci[:], pz0[:, 0:1], None, AT.is_equal)

    # ---- zero guard columns (written once; ops below write centers only) ----
    GP.memset(gates[:, :, :, 0:C0], 0.0)
    GP.memset(gates[:, :, :, C0 + W : WG], 0.0)
    for t in (sA, sB, sC, rA):
        GP.memset(t[:, :, 0:C0], 0.0)
        GP.memset(t[:, :, C0 + W : WG], 0.0)

    # ---- loads: depth-3 pipeline over 3 rotating stage buffers ----
    load_i = 0

    def load(dram_plane, cast_eng, dst):
        nonlocal load_i
        st = stages[load_i % 4]
        load_i += 1
        nc.sync.dma_start(out=st[:], in_=dram_plane.rearrange("(p r) j -> p r j", p=P))
        if cast_eng is V:
            V.tensor_copy(dst, st[:])
        else:
            ACT.activation(dst, st[:], AF.Copy)

    for b in range(BPC):
        load(d_dram[b, 0], ACT, rA[:, 4 * b : 4 * b + 4, C0 : C0 + W])

    # abs ping-pong: even k -> sA, odd k -> sB (product scratches, free here)
    def _abs(k):
        dst = sA if k % 2 == 0 else sB
        ACT.activation(_c(dst), gv(k), AF.Abs)
        return dst

    for k in range(K):
        eng = ACT if k < 4 else V
        for b in range(BPC):
            load(g_dram[b, k], eng, gates[:, k, 4 * b : 4 * b + 4, C0 : C0 + W])
        s = _abs(k)
        if k == 1:
            V.tensor_add(upA[:], _w(sA, 1), _c(sB))
            V.tensor_add(upB[:], gv(0, 1), gv(1))
        elif k == 2:
            V.tensor_add(upA[:], upA[:], _w(sA, -1))
            V.tensor_add(upB[:], upB[:], gv(2, -1))
        elif k == 4:
            V.tensor_add(u0A[:], _w(sB, 1), _w(sA, -1))
            V.tensor_add(u0B[:], gv(3, 1), gv(4, -1))
        elif k == 6:
            V.tensor_add(umA[:], _w(sB, 1), _c(sA))
            V.tensor_add(umB[:], gv(5, 1), gv(6))
        elif k == 7:
            V.tensor_add(umA[:], umA[:], _w(sB, -1))
            V.tensor_add(umB[:], umB[:], gv(7, -1))

    def combine(dst, up_t, u0_t, um_t):
        """dst[q] = up[q+1] + u0[q] + um[q-1] per image (q in 0..3).

        Partition-edge rows (q=0 and q=3) are built ENTIRELY in PSUM: the
        cross-partition halo term via the wdn/wup shift matmul plus the two
        in-partition terms via identity matmuls accumulating into the same
        bank; ACT drains then write those rows of dst directly.  DVE adds
        only the interior rows (q=1,2).  GPSIMD is never used: its tensor
        ucode both runs ~3x below DVE and starves DVE's SBUF ports ~4x
        while active (measured), so everything elementwise stays on DVE.
        """
        for b in range(BPC):
            PE.matmul(out=psum_dn[:, b, :], lhsT=wdn[:], rhs=up_t[:, 4 * b, :],
                      start=True, stop=False)
            PE.matmul(out=psum_dn[:, b, :], lhsT=wid[:], rhs=u0_t[:, 4 * b + 3, :],
                      start=False, stop=False)
            PE.matmul(out=psum_dn[:, b, :], lhsT=wid[:], rhs=um_t[:, 4 * b + 2, :],
                      start=False, stop=True)
            PE.matmul(out=psum_up[:, b, :], lhsT=wup[:], rhs=um_t[:, 4 * b + 3, :],
                      start=True, stop=False)
            PE.matmul(out=psum_up[:, b, :], lhsT=wid[:], rhs=u0_t[:, 4 * b, :],
                      start=False, stop=False)
            PE.matmul(out=psum_up[:, b, :], lhsT=wid[:], rhs=up_t[:, 4 * b + 1, :],
                      start=False, stop=True)
        ACT.activation(dst[:, 3::RPP, :], psum_dn[:], AF.Copy)
        ACT.activation(dst[:, 0::RPP, :], psum_up[:], AF.Copy)
        for b in range(BPC):
            V.tensor_add(dst[:, 4 * b + 1 : 4 * b + 3, :],
                         up_t[:, 4 * b + 2 : 4 * b + 4, :],
                         u0_t[:, 4 * b + 1 : 4 * b + 3, :])
        for b in range(BPC):
            V.tensor_add(dst[:, 4 * b + 1 : 4 * b + 3, :],
                         dst[:, 4 * b + 1 : 4 * b + 3, :],
                         um_t[:, 4 * b : 4 * b + 2, :])

    # ---- absw -> inv = exp(-ln(absw)); combine lands in `bias` scratch ----
    combine(bias, upA, u0A, umA)
    for b in range(BPC):
        ACT.activation(stages[b][:], bias[:, 4 * b : 4 * b + 4, :], AF.Ln)
        ACT.activation(inv[:, 4 * b : 4 * b + 4, :], stages[b][:], AF.Exp, scale=-1.0)

    # ---- gate_sum -> bias = raw - gate_sum*raw; combine lands in u0A ----
    combine(u0A, upB, u0B, umB)
    V.tensor_mul(u0A[:], u0A[:], inv[:])
    V.tensor_mul(u0A[:], u0A[:], _c(rA))
    V.tensor_sub(bias[:], _c(rA), u0A[:])

    # ---- propagation, r updated in place in rA ----
    # Class order up, u0, um: the psum_dn group's first two matmuls can then
    # start before the um tree finishes, keeping the PE+drain latency for the
    # edge rows off the DVE critical path.
    for step in range(PROP_TIME):
        V.tensor_mul(_c(sA), gv(0), _c(rA))
        V.tensor_mul(_c(sB), gv(1), _c(rA))
        V.tensor_mul(_c(sC), gv(2), _c(rA))
        V.tensor_add(upA[:], _w(sA, 1), _c(sB))
        V.tensor_add(upA[:], upA[:], _w(sC, -1))
        V.tensor_mul(_c(sA), gv(3), _c(rA))
        V.tensor_mul(_c(sB), gv(4), _c(rA))
        V.tensor_add(u0A[:], _w(sA, 1), _w(sB, -1))
        V.tensor_mul(_c(sA), gv(5), _c(rA))
        V.tensor_mul(_c(sB), gv(6), _c(rA))
        V.tensor_mul(_c(sC), gv(7), _c(rA))
        V.tensor_add(umA[:], _w(sA, 1), _c(sB))
        V.tensor_add(umA[:], umA[:], _w(sC, -1))
        combine(upB, upA, u0A, umA)
        if step < PROP_TIME - 1:
            V.tensor_mul(upB[:], upB[:], inv[:])
            V.tensor_add(_c(rA), upB[:], bias[:])
        else:
            V.tensor_mul(upB[:], upB[:], inv[:])
            V.tensor_add(_c(rA), upB[:], bias[:])
            for b in range(BPC):
                st = stages[b]
                V.tensor_copy(st[:], rA[:, 4 * b : 4 * b + 4, C0 : C0 + W])
                nc.sync.dma_start(
                    out=o_dram[b, 0].rearrange("(p r) j -> p r j", p=P), in_=st[:]
                )


def build(legalize=True):
    nc = bass.Bass()
    g_dram = nc.declare_dram_parameter("guidance", [BPC, K, H, W], F32, isOutput=False)
    d_dram = nc.declare_dram_parameter("blur_depth", [BPC, 1, H, W], F32, isOutput=False)
    o_dram = nc.declare_dram_parameter("out", [BPC, 1, H, W], F32, isOutput=True)
    with tile.TileContext(nc) as tc:
        with tc.tile_pool(name="main", bufs=1) as pool:
            with tc.tile_pool(name="ps", space="PSUM", bufs=1) as psum:
                _emit(nc, pool, psum, g_dram, d_dram, o_dram)
    if legalize:
        _split_excess_waits(nc)
    return nc


_NC = None


def _get_nc():
    global _NC
    if _NC is None:
        _NC = build()
    return _NC


def run(guidance, blur_depth, **spmd_kwargs):
    nc = _get_nc()
    in_maps = [
        {
            "guidance": np.ascontiguousarray(guidance[BPC * c : BPC * (c + 1)]),
            "blur_depth": np.ascontiguousarray(blur_depth[BPC * c : BPC * (c + 1)]),
        }
        for c in range(N_CORES)
    ]
    res = run_bass_kernel_spmd(nc, in_maps, list(range(N_CORES)), **spmd_kwargs)
    out = np.concatenate([res.results[i]["out"] for i in range(N_CORES)], axis=0)
    return out, res


def kernel(guidance, blur_depth):
    out, _ = run(guidance, blur_depth)
    return out.astype(np.float32)


# revision 25
# speedup vs baseline: 1.9537x; 1.0755x over previous
"""Affinity-propagation spatial stencil kernel for Trainium2 (8 NeuronCores).

Data-parallel: 16 images sharded 2-per-core; a core's 2 images are merged
into the free dimension as 8 flattened rows-per-partition ([P, 8, W]:
rows 4b..4b+3 belong to image b), so every engine op uses a 2-level
free access pattern (the DVE 2x fp16 mode and the GPSIMD ucode both
degrade on deeper APs).

Math (A_k = zero-padded shift by OFFSETS[k]; G_k = guidance channel k):
  absw = sum_k A_k |G_k|;  inv = 1/absw = exp(-ln(absw))
  gate_sum = (sum_k A_k G_k) * inv;  bias = raw - gate_sum * raw
  step:  r' = inv * (sum_k A_k (G_k * r)) + bias
(A_k G_k)*(A_k r) = A_k (G_k * r): products are unshifted muls; only the
shift-SUM moves data.  Column shifts ride free-dim AP offsets (guard
columns); row shifts act within a partition's 4 rows except the
partition-crossing row, which the idle TensorEngine produces as a matmul
with a sub/super-diagonal 0/1 stationary into PSUM (halo_dn[m] =
up[m+1, row0]).  ACT drains PSUM to SBUF fp16 (DVE reading PSUM directly
measured ~10x slow); the edge adds then run on fp16 in SBUF.

Per-step engine split: DVE products g0..g6 + up/u0 trees + combine +
inv/bias; GPSIMD the g7 product + um tree + one combine row-add; ACT the
2 PSUM drains; PE 4 halo matmuls.  Setup streams loads through 3
rotating stage buffers (depth-3 pipeline ~ HBM bound), converts on
ACT (k<4) / DVE (k>=4), abs on ACT, and runs the absw and gate-sum trees
behind the loads.
"""

import sys

sys.path.insert(0, "/opt/trn_rl_repo")

import numpy as np

import concourse.bass as bass
import concourse.mybir as mybir
from concourse import tile
from concourse.bass_utils import run_bass_kernel_spmd

N_CORES = 8
B, K, H, W = 16, 8, 512, 512
BPC = B // N_CORES  # images per core (merged: 8 rows per partition)
P = 128
RPP = H // P  # rows per partition per image
R2 = BPC * RPP  # flattened rows per partition
WG = W + 4  # guarded row width (image cols at [2:514])
C0 = 2
PROP_TIME = 4

F32 = mybir.dt.float32
DT = mybir.dt.float16
AT = mybir.AluOpType
AF = mybir.ActivationFunctionType


def _split_excess_waits(nc):
    """This walrus build encodes at most 1 sem wait per instruction; move the
    overflow onto preceding NoOps. Also drop EVENT_SEMAPHORE_RANGE_CLEAR
    (unencodable here; only appears at the kernel tail where it's a no-op)."""
    for f in nc.m.functions:
        for bb in f.blocks:
            new_insts = []
            for ins in bb.instructions:
                if getattr(ins, "op_name", None) == "EVENT_SEMAPHORE_RANGE_CLEAR":
                    continue
                cap = 1
                si = getattr(ins, "sync_info", None)
                if si is not None and si.on_wait and len(si.on_wait) > cap:
                    extra = list(si.on_wait[cap:])
                    del si.on_wait[cap:]
                    while extra:
                        nop = mybir.InstNoOp(
                            name=nc.get_next_instruction_name(),
                            engine=ins.engine,
                            sync_info=mybir.SyncInfo(on_wait=extra[:cap], on_update=[]),
                        )
                        new_insts.append(nop)
                        extra = extra[cap:]
                new_insts.append(ins)
            bb.instructions[:] = new_insts


def _c(ap):
    """center (image) view of a guarded [P, R2, WG] tile."""
    return ap[:, :, C0 : C0 + W]


def _w(ap, dj):
    """column-shifted view of a guarded tile: value at [i, j+dj]."""
    return ap[:, :, C0 + dj : C0 + dj + W]


def _emit(nc, pool, psum, g_dram, d_dram, o_dram):
    V = nc.vector
    GP = nc.gpsimd
    ACT = nc.scalar
    PE = nc.tensor

    def gtile(name):  # guarded work tile
        return pool.tile([P, R2, WG], DT, name=name)

    def utile(name):  # unguarded work tile
        return pool.tile([P, R2, W], DT, name=name)

    gates = pool.tile([P, K, R2, WG], DT, name="gates")
    stages = [pool.tile([P, RPP, W], F32, name=f"stg{i}") for i in range(4)]
    # sA/sB: abs ping-pong (setup); tmp1/tmp2: product temps (step-1 stream
    # during the load window, then every prop step -- each product is
    # consumed by the immediately following tree add, so two suffice).
    sA, sB = gtile("sA"), gtile("sB")
    tmp1, tmp2 = gtile("tmp1"), gtile("tmp2")
    # A-set: absw trees (streamed), then gate-sum trees, then step 2..4 trees.
    upA, u0A, umA = utile("upA"), utile("u0A"), utile("umA")
    # C-set: step-1 class sums (streamed during loads); step 2..4 combine dest.
    upC, u0C, umC = utile("upC"), utile("u0C"), utile("umC")
    rA = gtile("rA")
    inv = utile("inv")
    bias = utile("bias")
    # shift matrices for the PE halo: halo_dn[m]=x[m+1], halo_up[m]=x[m-1];
    # wid = identity (accumulates in-partition rows into the same PSUM bank)
    wdn = pool.tile([P, P], DT, name="wdn")
    wup = pool.tile([P, P], DT, name="wup")
    wid = pool.tile([P, P], DT, name="wid")
    ci = pool.tile([P, P], F32, name="ci")
    pm1 = pool.tile([P, 1], F32, name="pm1")
    pp1 = pool.tile([P, 1], F32, name="pp1")
    pz0 = pool.tile([P, 1], F32, name="pz0")
    psum_dn = psum.tile([P, BPC, W], F32, name="psum_dn", bufs=2)
    psum_up = psum.tile([P, BPC, W], F32, name="psum_up", bufs=2)

    def gv(k, dj=0):  # column-shifted gate view [P, R2, W]
        return gates[:, k, :, C0 + dj : C0 + dj + W]

    # ---- constants: shift matrices via iota + is_equal ----
    GP.iota(ci[:], [[1, P]], base=0, channel_multiplier=0,
            allow_small_or_imprecise_dtypes=True)  # ci[p,j] = j
    GP.iota(pm1[:], [[1, 1]], base=-1, channel_multiplier=1,
            allow_small_or_imprecise_dtypes=True)  # p-1
    GP.iota(pp1[:], [[1, 1]], base=1, channel_multiplier=1,
            allow_small_or_imprecise_dtypes=True)  # p+1
    GP.iota(pz0[:], [[1, 1]], base=0, channel_multiplier=1,
            allow_small_or_imprecise_dtypes=True)  # p
    # wdn[p,m] = 1 iff p == m+1  <=>  m == p-1 ; wup[p,m] = 1 iff m == p+1
    V.tensor_scalar(wdn[:], ci[:], pm1[:, 0:1], None, AT.is_equal)
    V.tensor_scalar(wup[:], ci[:], pp1[:, 0:1], None, AT.is_equal)
    V.tensor_scalar(wid[:], ci[:], pz0[:, 0:1], None, AT.is_equal)

    # ---- zero guard columns (written once; ops below write centers only) ----
    GP.memset(gates[:, :, :, 0:C0], 0.0)
    GP.memset(gates[:, :, :, C0 + W : WG], 0.0)
    for t in (sA, sB, tmp1, tmp2, rA):
        GP.memset(t[:, :, 0:C0], 0.0)
        GP.memset(t[:, :, C0 + W : WG], 0.0)

    # ---- loads: depth-3 pipeline over 3 rotating stage buffers ----
    load_i = 0

    def load(dram_plane, cast_eng, dst):
        nonlocal load_i
        st = stages[load_i % 4]
        load_i += 1
        nc.sync.dma_start(out=st[:], in_=dram_plane.rearrange("(p r) j -> p r j", p=P))
        if cast_eng is V:
            V.tensor_copy(dst, st[:])
        else:
            ACT.activation(dst, st[:], AF.Copy)

    for b in range(BPC):
        load(d_dram[b, 0], ACT, rA[:, 4 * b : 4 * b + 4, C0 : C0 + W])

    # abs ping-pong: even k -> sA, odd k -> sB (product scratches, free here)
    def _abs(k):
        dst = sA if k % 2 == 0 else sB
        ACT.activation(_c(dst), gv(k), AF.Abs)
        return dst

    # Streamed per gate k: loads + cast, |g_k| (ACT), step-1 product
    # g_k * raw (DVE, into tmp1/tmp2), and both the absw tree (A-set) and
    # step-1 tree (C-set) as their operands complete.
    for k in range(K):
        eng = ACT if k < 4 else V
        for b in range(BPC):
            load(g_dram[b, k], eng, gates[:, k, 4 * b : 4 * b + 4, C0 : C0 + W])
        _abs(k)
        t = tmp1 if k % 2 == 0 else tmp2
        V.tensor_mul(_c(t), gv(k), _c(rA))
        if k == 1:
            V.tensor_add(upC[:], _w(tmp1, 1), _c(tmp2))
            V.tensor_add(upA[:], _w(sA, 1), _c(sB))
        elif k == 2:
            V.tensor_add(upC[:], upC[:], _w(tmp1, -1))
            V.tensor_add(upA[:], upA[:], _w(sA, -1))
        elif k == 4:
            V.tensor_add(u0C[:], _w(tmp2, 1), _w(tmp1, -1))
            V.tensor_add(u0A[:], _w(sB, 1), _w(sA, -1))
        elif k == 6:
            V.tensor_add(umC[:], _w(tmp2, 1), _c(tmp1))
            V.tensor_add(umA[:], _w(sB, 1), _c(sA))
        elif k == 7:
            V.tensor_add(umC[:], umC[:], _w(tmp2, -1))
            V.tensor_add(umA[:], umA[:], _w(sB, -1))

    def combine(dst, up_t, u0_t, um_t):
        """dst[q] = up[q+1] + u0[q] + um[q-1] per image (q in 0..3).

        Partition-edge rows (q=0 and q=3) are built ENTIRELY in PSUM: the
        cross-partition halo term via the wdn/wup shift matmul plus the two
        in-partition terms via identity matmuls accumulating into the same
        bank; ACT drains then write those rows of dst directly.  DVE adds
        only the interior rows (q=1,2).  GPSIMD is never used: its tensor
        ucode both runs ~3x below DVE and starves DVE's SBUF ports ~4x
        while active (measured), so everything elementwise stays on DVE.
        """
        for b in range(BPC):
            PE.matmul(out=psum_dn[:, b, :], lhsT=wdn[:], rhs=up_t[:, 4 * b, :],
                      start=True, stop=False)
            PE.matmul(out=psum_dn[:, b, :], lhsT=wid[:], rhs=u0_t[:, 4 * b + 3, :],
                      start=False, stop=False)
            PE.matmul(out=psum_dn[:, b, :], lhsT=wid[:], rhs=um_t[:, 4 * b + 2, :],
                      start=False, stop=True)
            PE.matmul(out=psum_up[:, b, :], lhsT=wup[:], rhs=um_t[:, 4 * b + 3, :],
                      start=True, stop=False)
            PE.matmul(out=psum_up[:, b, :], lhsT=wid[:], rhs=u0_t[:, 4 * b, :],
                      start=False, stop=False)
            PE.matmul(out=psum_up[:, b, :], lhsT=wid[:], rhs=up_t[:, 4 * b + 1, :],
                      start=False, stop=True)
        ACT.activation(dst[:, 3::RPP, :], psum_dn[:], AF.Copy)
        ACT.activation(dst[:, 0::RPP, :], psum_up[:], AF.Copy)
        for b in range(BPC):
            V.tensor_add(dst[:, 4 * b + 1 : 4 * b + 3, :],
                         up_t[:, 4 * b + 2 : 4 * b + 4, :],
                         u0_t[:, 4 * b + 1 : 4 * b + 3, :])
        for b in range(BPC):
            V.tensor_add(dst[:, 4 * b + 1 : 4 * b + 3, :],
                         dst[:, 4 * b + 1 : 4 * b + 3, :],
                         um_t[:, 4 * b : 4 * b + 2, :])

    # ---- absw -> inv = exp(-ln(absw)); combine lands in `bias` scratch ----
    combine(bias, upA, u0A, umA)
    for b in range(BPC):
        ACT.activation(stages[b][:], bias[:, 4 * b : 4 * b + 4, :], AF.Ln)
        ACT.activation(inv[:, 4 * b : 4 * b + 4, :], stages[b][:], AF.Exp, scale=-1.0)

    # ---- step-1 shift-sum (inputs streamed above); lands in tmp1 center ----
    combine(_c(tmp1), upC, u0C, umC)

    # ---- gate_sum (A-set reused) -> bias = raw - gate_sum*inv*raw ----
    V.tensor_add(upA[:], gv(0, 1), gv(1))
    V.tensor_add(upA[:], upA[:], gv(2, -1))
    V.tensor_add(u0A[:], gv(3, 1), gv(4, -1))
    V.tensor_add(umA[:], gv(5, 1), gv(6))
    V.tensor_add(umA[:], umA[:], gv(7, -1))
    combine(_c(tmp2), upA, u0A, umA)
    V.tensor_mul(_c(tmp2), _c(tmp2), _c(rA))  # gate_sum_unnorm * raw
    V.tensor_mul(_c(tmp2), _c(tmp2), inv[:])
    V.tensor_sub(bias[:], _c(rA), _c(tmp2))

    # ---- finish step 1: r1 = inv * U1 + bias ----
    V.tensor_mul(_c(tmp1), _c(tmp1), inv[:])
    V.tensor_add(_c(rA), _c(tmp1), bias[:])

    def norm_split(acc, last):
        """r = inv*acc + bias, interior rows first (edge rows arrive late
        via PE+drain), then the stepped edge rows."""
        for b in range(BPC):
            V.tensor_mul(acc[:, 4 * b + 1 : 4 * b + 3, :],
                         acc[:, 4 * b + 1 : 4 * b + 3, :],
                         inv[:, 4 * b + 1 : 4 * b + 3, :])
            V.tensor_add(rA[:, 4 * b + 1 : 4 * b + 3, C0 : C0 + W],
                         acc[:, 4 * b + 1 : 4 * b + 3, :],
                         bias[:, 4 * b + 1 : 4 * b + 3, :])
        for q0 in (3, 0):
            V.tensor_mul(acc[:, q0::RPP, :], acc[:, q0::RPP, :], inv[:, q0::RPP, :])
            V.tensor_add(rA[:, q0::RPP, C0 : C0 + W], acc[:, q0::RPP, :],
                         bias[:, q0::RPP, :])
        if last:
            for b in range(BPC):
                st = stages[b]
                V.tensor_copy(st[:], rA[:, 4 * b : 4 * b + 4, C0 : C0 + W])
                nc.sync.dma_start(
                    out=o_dram[b, 0].rearrange("(p r) j -> p r j", p=P), in_=st[:]
                )

    # ---- steps 2..4, r updated in place in rA ----
    # Class order up, u0, um: the psum groups' first two matmuls start
    # before the um tree finishes, so only the um-dependent matmul + drain
    # remain on the tail; norm_split hides that behind the interior rows.
    for step in range(1, PROP_TIME):
        V.tensor_mul(_c(tmp1), gv(0), _c(rA))
        V.tensor_mul(_c(tmp2), gv(1), _c(rA))
        V.tensor_add(upA[:], _w(tmp1, 1), _c(tmp2))
        V.tensor_mul(_c(tmp1), gv(2), _c(rA))
        V.tensor_add(upA[:], upA[:], _w(tmp1, -1))
        V.tensor_mul(_c(tmp2), gv(3), _c(rA))
        V.tensor_mul(_c(tmp1), gv(4), _c(rA))
        V.tensor_add(u0A[:], _w(tmp2, 1), _w(tmp1, -1))
        V.tensor_mul(_c(tmp2), gv(5), _c(rA))
        V.tensor_mul(_c(tmp1), gv(6), _c(rA))
        V.tensor_add(umA[:], _w(tmp2, 1), _c(tmp1))
        V.tensor_mul(_c(tmp2), gv(7), _c(rA))
        V.tensor_add(umA[:], umA[:], _w(tmp2, -1))
        combine(upC, upA, u0A, umA)
        norm_split(upC, last=(step == PROP_TIME - 1))


def build(legalize=True):
    nc = bass.Bass()
    g_dram = nc.declare_dram_parameter("guidance", [BPC, K, H, W], F32, isOutput=False)
    d_dram = nc.declare_dram_parameter("blur_depth", [BPC, 1, H, W], F32, isOutput=False)
    o_dram = nc.declare_dram_parameter("out", [BPC, 1, H, W], F32, isOutput=True)
    with tile.TileContext(nc) as tc:
        with tc.tile_pool(name="main", bufs=1) as pool:
            with tc.tile_pool(name="ps", space="PSUM", bufs=1) as psum:
                _emit(nc, pool, psum, g_dram, d_dram, o_dram)
    if legalize:
        _split_excess_waits(nc)
    return nc


_NC = None


def _get_nc():
    global _NC
    if _NC is None:
        _NC = build()
    return _NC


def run(guidance, blur_depth, **spmd_kwargs):
    nc = _get_nc()
    in_maps = [
        {
            "guidance": np.ascontiguousarray(guidance[BPC * c : BPC * (c + 1)]),
            "blur_depth": np.ascontiguousarray(blur_depth[BPC * c : BPC * (c + 1)]),
        }
        for c in range(N_CORES)
    ]
    res = run_bass_kernel_spmd(nc, in_maps, list(range(N_CORES)), **spmd_kwargs)
    out = np.concatenate([res.results[i]["out"] for i in range(N_CORES)], axis=0)
    return out, res


def kernel(guidance, blur_depth):
    out, _ = run(guidance, blur_depth)
    return out.astype(np.float32)


# revision 26
# speedup vs baseline: 1.9897x; 1.0184x over previous
"""Affinity-propagation spatial stencil kernel for Trainium2 (8 NeuronCores).

Data-parallel: 16 images sharded 2-per-core; a core's 2 images are merged
into the free dimension as 8 flattened rows-per-partition ([P, 8, W]:
rows 4b..4b+3 belong to image b), so every engine op uses a 2-level
free access pattern (the DVE 2x fp16 mode and the GPSIMD ucode both
degrade on deeper APs).

Math (A_k = zero-padded shift by OFFSETS[k]; G_k = guidance channel k):
  absw = sum_k A_k |G_k|;  inv = 1/absw = exp(-ln(absw))
  gate_sum = (sum_k A_k G_k) * inv;  bias = raw - gate_sum * raw
  step:  r' = inv * (sum_k A_k (G_k * r)) + bias
(A_k G_k)*(A_k r) = A_k (G_k * r): products are unshifted muls; only the
shift-SUM moves data.  Column shifts ride free-dim AP offsets (guard
columns); row shifts act within a partition's 4 rows except the
partition-crossing row, which the idle TensorEngine produces as a matmul
with a sub/super-diagonal 0/1 stationary into PSUM (halo_dn[m] =
up[m+1, row0]).  ACT drains PSUM to SBUF fp16 (DVE reading PSUM directly
measured ~10x slow); the edge adds then run on fp16 in SBUF.

Per-step engine split: DVE products g0..g6 + up/u0 trees + combine +
inv/bias; GPSIMD the g7 product + um tree + one combine row-add; ACT the
2 PSUM drains; PE 4 halo matmuls.  Setup streams loads through 3
rotating stage buffers (depth-3 pipeline ~ HBM bound), converts on
ACT (k<4) / DVE (k>=4), abs on ACT, and runs the absw and gate-sum trees
behind the loads.
"""

import sys

sys.path.insert(0, "/opt/trn_rl_repo")

import numpy as np

import concourse.bass as bass
import concourse.mybir as mybir
from concourse import tile
from concourse.bass_utils import run_bass_kernel_spmd

N_CORES = 8
B, K, H, W = 16, 8, 512, 512
BPC = B // N_CORES  # images per core (merged: 8 rows per partition)
P = 128
RPP = H // P  # rows per partition per image
R2 = BPC * RPP  # flattened rows per partition
WG = W + 4  # guarded row width (image cols at [2:514])
C0 = 2
PROP_TIME = 4

F32 = mybir.dt.float32
DT = mybir.dt.float16
AT = mybir.AluOpType
AF = mybir.ActivationFunctionType


def _split_excess_waits(nc):
    """This walrus build encodes at most 1 sem wait per instruction; move the
    overflow onto preceding NoOps. Also drop EVENT_SEMAPHORE_RANGE_CLEAR
    (unencodable here; only appears at the kernel tail where it's a no-op)."""
    for f in nc.m.functions:
        for bb in f.blocks:
            new_insts = []
            for ins in bb.instructions:
                if getattr(ins, "op_name", None) == "EVENT_SEMAPHORE_RANGE_CLEAR":
                    continue
                cap = 1
                si = getattr(ins, "sync_info", None)
                if si is not None and si.on_wait and len(si.on_wait) > cap:
                    extra = list(si.on_wait[cap:])
                    del si.on_wait[cap:]
                    while extra:
                        nop = mybir.InstNoOp(
                            name=nc.get_next_instruction_name(),
                            engine=ins.engine,
                            sync_info=mybir.SyncInfo(on_wait=extra[:cap], on_update=[]),
                        )
                        new_insts.append(nop)
                        extra = extra[cap:]
                new_insts.append(ins)
            bb.instructions[:] = new_insts


def _c(ap):
    """center (image) view of a guarded [P, R2, WG] tile."""
    return ap[:, :, C0 : C0 + W]


def _w(ap, dj):
    """column-shifted view of a guarded tile: value at [i, j+dj]."""
    return ap[:, :, C0 + dj : C0 + dj + W]


def _emit(nc, pool, psum, g_dram, d_dram, o_dram):
    V = nc.vector
    GP = nc.gpsimd
    ACT = nc.scalar
    PE = nc.tensor

    def gtile(name):  # guarded work tile
        return pool.tile([P, R2, WG], DT, name=name)

    def utile(name):  # unguarded work tile
        return pool.tile([P, R2, W], DT, name=name)

    gates = pool.tile([P, K, R2, WG], DT, name="gates")
    stages = [pool.tile([P, RPP, W], F32, name=f"stg{i}") for i in range(4)]
    # sA/sB: abs ping-pong (setup); tmp1/tmp2: product temps (step-1 stream
    # during the load window, then every prop step -- each product is
    # consumed by the immediately following tree add, so two suffice).
    sA, sB = gtile("sA"), gtile("sB")
    tmp1, tmp2 = gtile("tmp1"), gtile("tmp2")
    # A-set: absw trees (streamed), then gate-sum trees, then step 2..4 trees.
    upA, u0A, umA = utile("upA"), utile("u0A"), utile("umA")
    # C-set: step-1 class sums (streamed during loads); step 2..4 combine dest.
    upC, u0C, umC = utile("upC"), utile("u0C"), utile("umC")
    rA = gtile("rA")
    inv = utile("inv")
    bias = utile("bias")
    # shift matrices for the PE halo: halo_dn[m]=x[m+1], halo_up[m]=x[m-1];
    # wid = identity (accumulates in-partition rows into the same PSUM bank)
    wdn = pool.tile([P, P], DT, name="wdn")
    wup = pool.tile([P, P], DT, name="wup")
    wid = pool.tile([P, P], DT, name="wid")
    ci = pool.tile([P, P], F32, name="ci")
    pm1 = pool.tile([P, 1], F32, name="pm1")
    pp1 = pool.tile([P, 1], F32, name="pp1")
    pz0 = pool.tile([P, 1], F32, name="pz0")
    psum_dn = psum.tile([P, BPC, W], F32, name="psum_dn", bufs=2)
    psum_up = psum.tile([P, BPC, W], F32, name="psum_up", bufs=2)

    def gv(k, dj=0):  # column-shifted gate view [P, R2, W]
        return gates[:, k, :, C0 + dj : C0 + dj + W]

    # ---- loads: depth-4 pipeline over 4 rotating stage buffers; the first
    # triggers are emitted before any constants so DMA starts immediately ----
    load_i = 0

    def load(dram_plane, cast_eng, dst):
        nonlocal load_i
        st = stages[load_i % 4]
        load_i += 1
        nc.sync.dma_start(out=st[:], in_=dram_plane.rearrange("(p r) j -> p r j", p=P))
        if cast_eng is V:
            V.tensor_copy(dst, st[:])
        else:
            ACT.activation(dst, st[:], AF.Copy)

    for b in range(BPC):
        load(d_dram[b, 0], V, rA[:, 4 * b : 4 * b + 4, C0 : C0 + W])

    # ---- constants: shift matrices via iota + is_equal ----
    GP.iota(ci[:], [[1, P]], base=0, channel_multiplier=0,
            allow_small_or_imprecise_dtypes=True)  # ci[p,j] = j
    GP.iota(pm1[:], [[1, 1]], base=-1, channel_multiplier=1,
            allow_small_or_imprecise_dtypes=True)  # p-1
    GP.iota(pp1[:], [[1, 1]], base=1, channel_multiplier=1,
            allow_small_or_imprecise_dtypes=True)  # p+1
    GP.iota(pz0[:], [[1, 1]], base=0, channel_multiplier=1,
            allow_small_or_imprecise_dtypes=True)  # p
    # wdn[p,m] = 1 iff p == m+1  <=>  m == p-1 ; wup[p,m] = 1 iff m == p+1
    V.tensor_scalar(wdn[:], ci[:], pm1[:, 0:1], None, AT.is_equal)
    V.tensor_scalar(wup[:], ci[:], pp1[:, 0:1], None, AT.is_equal)
    V.tensor_scalar(wid[:], ci[:], pz0[:, 0:1], None, AT.is_equal)

    # ---- zero guard columns (written once; ops below write centers only) ----
    GP.memset(gates[:, :, :, 0:C0], 0.0)
    GP.memset(gates[:, :, :, C0 + W : WG], 0.0)
    for t in (sA, sB, tmp1, tmp2, rA):
        GP.memset(t[:, :, 0:C0], 0.0)
        GP.memset(t[:, :, C0 + W : WG], 0.0)

    # abs ping-pong: even k -> sA, odd k -> sB (product scratches, free here)
    def _abs(k):
        dst = sA if k % 2 == 0 else sB
        ACT.activation(_c(dst), gv(k), AF.Abs)
        return dst

    # Streamed per gate k: loads + cast, |g_k| (ACT), step-1 product
    # g_k * raw (DVE, into tmp1/tmp2), and both the absw tree (A-set) and
    # step-1 tree (C-set) as their operands complete.
    for k in range(K):
        eng = ACT if k < 4 else V
        for b in range(BPC):
            load(g_dram[b, k], eng, gates[:, k, 4 * b : 4 * b + 4, C0 : C0 + W])
        _abs(k)
        t = tmp1 if k % 2 == 0 else tmp2
        V.tensor_mul(_c(t), gv(k), _c(rA))
        if k == 1:
            V.tensor_add(upC[:], _w(tmp1, 1), _c(tmp2))
            V.tensor_add(upA[:], _w(sA, 1), _c(sB))
        elif k == 2:
            V.tensor_add(upC[:], upC[:], _w(tmp1, -1))
            V.tensor_add(upA[:], upA[:], _w(sA, -1))
        elif k == 4:
            V.tensor_add(u0C[:], _w(tmp2, 1), _w(tmp1, -1))
            V.tensor_add(u0A[:], _w(sB, 1), _w(sA, -1))
        elif k == 6:
            V.tensor_add(umC[:], _w(tmp2, 1), _c(tmp1))
            V.tensor_add(umA[:], _w(sB, 1), _c(sA))
        elif k == 7:
            V.tensor_add(umC[:], umC[:], _w(tmp2, -1))
            V.tensor_add(umA[:], umA[:], _w(sB, -1))

    def combine(dst, up_t, u0_t, um_t):
        """dst[q] = up[q+1] + u0[q] + um[q-1] per image (q in 0..3).

        Partition-edge rows (q=0 and q=3) are built ENTIRELY in PSUM: the
        cross-partition halo term via the wdn/wup shift matmul plus the two
        in-partition terms via identity matmuls accumulating into the same
        bank; ACT drains then write those rows of dst directly.  DVE adds
        only the interior rows (q=1,2).  GPSIMD is never used: its tensor
        ucode both runs ~3x below DVE and starves DVE's SBUF ports ~4x
        while active (measured), so everything elementwise stays on DVE.
        """
        for b in range(BPC):
            PE.matmul(out=psum_dn[:, b, :], lhsT=wdn[:], rhs=up_t[:, 4 * b, :],
                      start=True, stop=False)
            PE.matmul(out=psum_dn[:, b, :], lhsT=wid[:], rhs=u0_t[:, 4 * b + 3, :],
                      start=False, stop=False)
            PE.matmul(out=psum_dn[:, b, :], lhsT=wid[:], rhs=um_t[:, 4 * b + 2, :],
                      start=False, stop=True)
            PE.matmul(out=psum_up[:, b, :], lhsT=wup[:], rhs=um_t[:, 4 * b + 3, :],
                      start=True, stop=False)
            PE.matmul(out=psum_up[:, b, :], lhsT=wid[:], rhs=u0_t[:, 4 * b, :],
                      start=False, stop=False)
            PE.matmul(out=psum_up[:, b, :], lhsT=wid[:], rhs=up_t[:, 4 * b + 1, :],
                      start=False, stop=True)
        ACT.activation(dst[:, 3::RPP, :], psum_dn[:], AF.Copy)
        ACT.activation(dst[:, 0::RPP, :], psum_up[:], AF.Copy)
        for b in range(BPC):
            V.tensor_add(dst[:, 4 * b + 1 : 4 * b + 3, :],
                         up_t[:, 4 * b + 2 : 4 * b + 4, :],
                         u0_t[:, 4 * b + 1 : 4 * b + 3, :])
        for b in range(BPC):
            V.tensor_add(dst[:, 4 * b + 1 : 4 * b + 3, :],
                         dst[:, 4 * b + 1 : 4 * b + 3, :],
                         um_t[:, 4 * b : 4 * b + 2, :])

    # ---- absw -> inv = exp(-ln(absw)); combine lands in `bias` scratch ----
    combine(bias, upA, u0A, umA)
    for b in range(BPC):
        ACT.activation(stages[b][:], bias[:, 4 * b : 4 * b + 4, :], AF.Ln)
        ACT.activation(inv[:, 4 * b : 4 * b + 4, :], stages[b][:], AF.Exp, scale=-1.0)

    # ---- step-1 shift-sum (inputs streamed above); lands in tmp1 center ----
    combine(_c(tmp1), upC, u0C, umC)

    # ---- gate_sum (A-set reused) -> bias = raw - gate_sum*inv*raw ----
    V.tensor_add(upA[:], gv(0, 1), gv(1))
    V.tensor_add(upA[:], upA[:], gv(2, -1))
    V.tensor_add(u0A[:], gv(3, 1), gv(4, -1))
    V.tensor_add(umA[:], gv(5, 1), gv(6))
    V.tensor_add(umA[:], umA[:], gv(7, -1))
    combine(_c(tmp2), upA, u0A, umA)
    V.tensor_mul(_c(tmp2), _c(tmp2), _c(rA))  # gate_sum_unnorm * raw
    V.tensor_mul(_c(tmp2), _c(tmp2), inv[:])
    V.tensor_sub(bias[:], _c(rA), _c(tmp2))

    # ---- finish step 1: r1 = inv * U1 + bias ----
    V.tensor_mul(_c(tmp1), _c(tmp1), inv[:])
    V.tensor_add(_c(rA), _c(tmp1), bias[:])

    def norm_split(acc, last):
        """r = inv*acc + bias, interior rows first (edge rows arrive late
        via PE+drain), then the stepped edge rows."""
        for b in range(BPC):
            V.tensor_mul(acc[:, 4 * b + 1 : 4 * b + 3, :],
                         acc[:, 4 * b + 1 : 4 * b + 3, :],
                         inv[:, 4 * b + 1 : 4 * b + 3, :])
            V.tensor_add(rA[:, 4 * b + 1 : 4 * b + 3, C0 : C0 + W],
                         acc[:, 4 * b + 1 : 4 * b + 3, :],
                         bias[:, 4 * b + 1 : 4 * b + 3, :])
        for q0 in (3, 0):
            V.tensor_mul(acc[:, q0::RPP, :], acc[:, q0::RPP, :], inv[:, q0::RPP, :])
            V.tensor_add(rA[:, q0::RPP, C0 : C0 + W], acc[:, q0::RPP, :],
                         bias[:, q0::RPP, :])
        if last:
            for b in range(BPC):
                st = stages[b]
                V.tensor_copy(st[:], rA[:, 4 * b : 4 * b + 4, C0 : C0 + W])
                nc.sync.dma_start(
                    out=o_dram[b, 0].rearrange("(p r) j -> p r j", p=P), in_=st[:]
                )

    # ---- steps 2..4, r updated in place in rA ----
    # Class order up, u0, um: the psum groups' first two matmuls start
    # before the um tree finishes, so only the um-dependent matmul + drain
    # remain on the tail; norm_split hides that behind the interior rows.
    for step in range(1, PROP_TIME):
        V.tensor_mul(_c(tmp1), gv(0), _c(rA))
        V.tensor_mul(_c(tmp2), gv(1), _c(rA))
        V.tensor_add(upA[:], _w(tmp1, 1), _c(tmp2))
        V.tensor_mul(_c(tmp1), gv(2), _c(rA))
        V.tensor_add(upA[:], upA[:], _w(tmp1, -1))
        V.tensor_mul(_c(tmp2), gv(3), _c(rA))
        V.tensor_mul(_c(tmp1), gv(4), _c(rA))
        V.tensor_add(u0A[:], _w(tmp2, 1), _w(tmp1, -1))
        V.tensor_mul(_c(tmp2), gv(5), _c(rA))
        V.tensor_mul(_c(tmp1), gv(6), _c(rA))
        V.tensor_add(umA[:], _w(tmp2, 1), _c(tmp1))
        V.tensor_mul(_c(tmp2), gv(7), _c(rA))
        V.tensor_add(umA[:], umA[:], _w(tmp2, -1))
        combine(upC, upA, u0A, umA)
        norm_split(upC, last=(step == PROP_TIME - 1))


def build(legalize=True):
    nc = bass.Bass()
    g_dram = nc.declare_dram_parameter("guidance", [BPC, K, H, W], F32, isOutput=False)
    d_dram = nc.declare_dram_parameter("blur_depth", [BPC, 1, H, W], F32, isOutput=False)
    o_dram = nc.declare_dram_parameter("out", [BPC, 1, H, W], F32, isOutput=True)
    with tile.TileContext(nc) as tc:
        with tc.tile_pool(name="main", bufs=1) as pool:
            with tc.tile_pool(name="ps", space="PSUM", bufs=1) as psum:
                _emit(nc, pool, psum, g_dram, d_dram, o_dram)
    if legalize:
        _split_excess_waits(nc)
    return nc


_NC = None


def _get_nc():
    global _NC
    if _NC is None:
        _NC = build()
    return _NC


def run(guidance, blur_depth, **spmd_kwargs):
    nc = _get_nc()
    in_maps = [
        {
            "guidance": np.ascontiguousarray(guidance[BPC * c : BPC * (c + 1)]),
            "blur_depth": np.ascontiguousarray(blur_depth[BPC * c : BPC * (c + 1)]),
        }
        for c in range(N_CORES)
    ]
    res = run_bass_kernel_spmd(nc, in_maps, list(range(N_CORES)), **spmd_kwargs)
    out = np.concatenate([res.results[i]["out"] for i in range(N_CORES)], axis=0)
    return out, res


def kernel(guidance, blur_depth):
    out, _ = run(guidance, blur_depth)
    return out.astype(np.float32)


# revision 31
# speedup vs baseline: 2.0605x; 1.0356x over previous
"""Affinity-propagation spatial stencil kernel for Trainium2 (8 NeuronCores).

Data-parallel: 16 images sharded 2-per-core; a core's 2 images are merged
into the free dimension as 8 flattened rows-per-partition ([P, 8, W]:
rows 4b..4b+3 belong to image b), so every engine op uses a 2-level
free access pattern (the DVE 2x fp16 mode and the GPSIMD ucode both
degrade on deeper APs).

Math (A_k = zero-padded shift by OFFSETS[k]; G_k = guidance channel k):
  absw = sum_k A_k |G_k|;  inv = 1/absw = exp(-ln(absw))
  gate_sum = (sum_k A_k G_k) * inv;  bias = raw - gate_sum * raw
  step:  r' = inv * (sum_k A_k (G_k * r)) + bias
(A_k G_k)*(A_k r) = A_k (G_k * r): products are unshifted muls; only the
shift-SUM moves data.  Column shifts ride free-dim AP offsets (guard
columns); row shifts act within a partition's 4 rows except the
partition-crossing row, which the idle TensorEngine produces as a matmul
with a sub/super-diagonal 0/1 stationary into PSUM (halo_dn[m] =
up[m+1, row0]).  ACT drains PSUM to SBUF fp16 (DVE reading PSUM directly
measured ~10x slow); the edge adds then run on fp16 in SBUF.

Per-step engine split: DVE products g0..g6 + up/u0 trees + combine +
inv/bias; GPSIMD the g7 product + um tree + one combine row-add; ACT the
2 PSUM drains; PE 4 halo matmuls.  Setup streams loads through 3
rotating stage buffers (depth-3 pipeline ~ HBM bound), converts on
ACT (k<4) / DVE (k>=4), abs on ACT, and runs the absw and gate-sum trees
behind the loads.
"""

import sys

sys.path.insert(0, "/opt/trn_rl_repo")

import numpy as np

import concourse.bass as bass
import concourse.mybir as mybir
from concourse import tile
from concourse.bass_utils import run_bass_kernel_spmd

N_CORES = 8
B, K, H, W = 16, 8, 512, 512
BPC = B // N_CORES  # images per core (merged: 8 rows per partition)
P = 128
RPP = H // P  # rows per partition per image
R2 = BPC * RPP  # flattened rows per partition
WG = W + 4  # guarded row width (image cols at [2:514])
C0 = 2
PROP_TIME = 4

F32 = mybir.dt.float32
DT = mybir.dt.float16
AT = mybir.AluOpType
AF = mybir.ActivationFunctionType


def _split_excess_waits(nc):
    """This walrus build encodes at most 1 sem wait per instruction; move the
    overflow onto preceding NoOps. Also drop EVENT_SEMAPHORE_RANGE_CLEAR
    (unencodable here; only appears at the kernel tail where it's a no-op)."""
    for f in nc.m.functions:
        for bb in f.blocks:
            new_insts = []
            for ins in bb.instructions:
                if getattr(ins, "op_name", None) == "EVENT_SEMAPHORE_RANGE_CLEAR":
                    continue
                cap = 1
                si = getattr(ins, "sync_info", None)
                if si is not None and si.on_wait and len(si.on_wait) > cap:
                    extra = list(si.on_wait[cap:])
                    del si.on_wait[cap:]
                    while extra:
                        nop = mybir.InstNoOp(
                            name=nc.get_next_instruction_name(),
                            engine=ins.engine,
                            sync_info=mybir.SyncInfo(on_wait=extra[:cap], on_update=[]),
                        )
                        new_insts.append(nop)
                        extra = extra[cap:]
                new_insts.append(ins)
            bb.instructions[:] = new_insts


def _c(ap):
    """center (image) view of a guarded [P, R2, WG] tile."""
    return ap[:, :, C0 : C0 + W]


def _w(ap, dj):
    """column-shifted view of a guarded tile: value at [i, j+dj]."""
    return ap[:, :, C0 + dj : C0 + dj + W]


def _emit(nc, pool, psum, g_dram, d_dram, o_dram):
    V = nc.vector
    GP = nc.gpsimd
    ACT = nc.scalar
    PE = nc.tensor

    def gtile(name):  # guarded work tile
        return pool.tile([P, R2, WG], DT, name=name)

    def utile(name):  # unguarded work tile
        return pool.tile([P, R2, W], DT, name=name)

    gates = pool.tile([P, K, R2, WG], DT, name="gates")
    stages = [pool.tile([P, RPP, W], F32, name=f"stg{i}") for i in range(4)]
    # sA/sB: abs ping-pong (setup); tmp1/tmp2: product temps (step-1 stream
    # during the load window, then every prop step -- each product is
    # consumed by the immediately following tree add, so two suffice).
    sA, sB = gtile("sA"), gtile("sB")
    tmp1, tmp2 = gtile("tmp1"), gtile("tmp2")
    # A-set: absw trees (streamed), then gate-sum trees, then step 2..4 trees.
    upA, u0A, umA = utile("upA"), utile("u0A"), utile("umA")
    # C-set: step-1 class sums (streamed during loads); step 2..4 combine dest.
    upC, u0C, umC = utile("upC"), utile("u0C"), utile("umC")
    rA = gtile("rA")
    inv = utile("inv")
    bias = utile("bias")
    # shift matrices for the PE halo: halo_dn[m]=x[m+1], halo_up[m]=x[m-1];
    # wid = identity (accumulates in-partition rows into the same PSUM bank)
    wdn = pool.tile([P, P], DT, name="wdn")
    wup = pool.tile([P, P], DT, name="wup")
    wid = pool.tile([P, P], DT, name="wid")
    ci = pool.tile([P, P], F32, name="ci")
    pm1 = pool.tile([P, 1], F32, name="pm1")
    pp1 = pool.tile([P, 1], F32, name="pp1")
    pz0 = pool.tile([P, 1], F32, name="pz0")
    psum_dn = psum.tile([P, BPC, W], F32, name="psum_dn", bufs=2)
    psum_up = psum.tile([P, BPC, W], F32, name="psum_up", bufs=2)

    def gv(k, dj=0):  # column-shifted gate view [P, R2, W]
        return gates[:, k, :, C0 + dj : C0 + dj + W]

    # ---- loads: depth-4 pipeline over 4 rotating stage buffers; the first
    # triggers are emitted before any constants so DMA starts immediately ----
    load_i = 0

    def load(dram_plane, cast_eng, dst):
        nonlocal load_i
        st = stages[load_i % 4]
        load_i += 1
        nc.sync.dma_start(out=st[:], in_=dram_plane.rearrange("(p r) j -> p r j", p=P))
        if cast_eng is V:
            V.tensor_copy(dst, st[:])
        else:
            ACT.activation(dst, st[:], AF.Copy)

    for b in range(BPC):
        load(d_dram[b, 0], V, rA[:, 4 * b : 4 * b + 4, C0 : C0 + W])

    # ---- constants: shift matrices via iota + is_equal ----
    GP.iota(ci[:], [[1, P]], base=0, channel_multiplier=0,
            allow_small_or_imprecise_dtypes=True)  # ci[p,j] = j
    GP.iota(pm1[:], [[1, 1]], base=-1, channel_multiplier=1,
            allow_small_or_imprecise_dtypes=True)  # p-1
    GP.iota(pp1[:], [[1, 1]], base=1, channel_multiplier=1,
            allow_small_or_imprecise_dtypes=True)  # p+1
    GP.iota(pz0[:], [[1, 1]], base=0, channel_multiplier=1,
            allow_small_or_imprecise_dtypes=True)  # p
    # wdn[p,m] = 1 iff p == m+1  <=>  m == p-1 ; wup[p,m] = 1 iff m == p+1
    V.tensor_scalar(wdn[:], ci[:], pm1[:, 0:1], None, AT.is_equal)
    V.tensor_scalar(wup[:], ci[:], pp1[:, 0:1], None, AT.is_equal)
    V.tensor_scalar(wid[:], ci[:], pz0[:, 0:1], None, AT.is_equal)

    # ---- zero guard columns (written once; ops below write centers only) ----
    GP.memset(gates[:, :, :, 0:C0], 0.0)
    GP.memset(gates[:, :, :, C0 + W : WG], 0.0)
    for t in (sA, sB, tmp1, tmp2, rA):
        GP.memset(t[:, :, 0:C0], 0.0)
        GP.memset(t[:, :, C0 + W : WG], 0.0)

    # abs ping-pong: even k -> sA, odd k -> sB (product scratches, free here)
    def _abs(k):
        dst = sA if k % 2 == 0 else sB
        ACT.activation(_c(dst), gv(k), AF.Abs)
        return dst

    # Streamed per gate k: loads + cast, |g_k| (ACT), step-1 product
    # g_k * raw (DVE, into tmp1/tmp2), and both the absw tree (A-set) and
    # step-1 tree (C-set) as their operands complete.
    for k in range(K):
        eng = ACT if k < 4 else V
        for b in range(BPC):
            load(g_dram[b, k], eng, gates[:, k, 4 * b : 4 * b + 4, C0 : C0 + W])
        _abs(k)
        t = tmp1 if k % 2 == 0 else tmp2
        V.tensor_mul(_c(t), gv(k), _c(rA))
        if k == 1:
            V.tensor_add(upC[:], _w(tmp1, 1), _c(tmp2))
            V.tensor_add(upA[:], _w(sA, 1), _c(sB))
        elif k == 2:
            V.tensor_add(upC[:], upC[:], _w(tmp1, -1))
            V.tensor_add(upA[:], upA[:], _w(sA, -1))
        elif k == 4:
            V.tensor_add(u0C[:], _w(tmp2, 1), _w(tmp1, -1))
            V.tensor_add(u0A[:], _w(sB, 1), _w(sA, -1))
        elif k == 6:
            V.tensor_add(umC[:], _w(tmp2, 1), _c(tmp1))
            V.tensor_add(umA[:], _w(sB, 1), _c(sA))
        elif k == 7:
            V.tensor_add(umC[:], umC[:], _w(tmp2, -1))
            V.tensor_add(umA[:], umA[:], _w(sB, -1))

    def combine(dst, up_t, u0_t, um_t, u0_after_um=False):
        """dst[q] = up[q+1] + u0[q] + um[q-1] per image (q in 0..3).
        u0_t: a plane, or a list of fns mapping a row slice to a
        column-shifted view (the u0 class folded into the PSUM groups and
        interior adds without materializing its own tree sum).

        Partition-edge rows (q=0 and q=3) are built ENTIRELY in PSUM: the
        cross-partition halo term via the wdn/wup shift matmul plus the two
        in-partition terms via identity matmuls accumulating into the same
        bank; ACT drains then write those rows of dst directly.  DVE adds
        only the interior rows (q=1,2).  GPSIMD is never used: its tensor
        ucode both runs ~3x below DVE and starves DVE's SBUF ports ~4x
        while active (measured), so everything elementwise stays on DVE.
        """
        u0_fns = u0_t if isinstance(u0_t, list) else [lambda rs, t=u0_t: t[:, rs, :]]
        for b in range(BPC):
            mms = [(wdn, up_t[:, 4 * b, :])]
            mms += [(wid, f(4 * b + 3)) for f in u0_fns]
            mms += [(wid, um_t[:, 4 * b + 2, :])]
            if u0_after_um:  # u0 operands finish last: keep them at the tail
                mms = [mms[0], mms[-1]] + mms[1:-1]
            for i, (wm, rhs) in enumerate(mms):
                PE.matmul(out=psum_dn[:, b, :], lhsT=wm[:], rhs=rhs,
                          start=(i == 0), stop=(i == len(mms) - 1))
            mms = [(wid, up_t[:, 4 * b + 1, :])]
            mms += [(wid, f(4 * b)) for f in u0_fns]
            mms += [(wup, um_t[:, 4 * b + 3, :])]
            if u0_after_um:
                mms = [mms[0], mms[-1]] + mms[1:-1]
            for i, (wm, rhs) in enumerate(mms):
                PE.matmul(out=psum_up[:, b, :], lhsT=wm[:], rhs=rhs,
                          start=(i == 0), stop=(i == len(mms) - 1))
        ACT.activation(dst[:, 3::RPP, :], psum_dn[:], AF.Copy)
        ACT.activation(dst[:, 0::RPP, :], psum_up[:], AF.Copy)
        for b in range(BPC):
            V.tensor_add(dst[:, 4 * b + 1 : 4 * b + 3, :],
                         up_t[:, 4 * b + 2 : 4 * b + 4, :],
                         u0_fns[0](slice(4 * b + 1, 4 * b + 3)))
            for f in u0_fns[1:]:
                V.tensor_add(dst[:, 4 * b + 1 : 4 * b + 3, :],
                             dst[:, 4 * b + 1 : 4 * b + 3, :],
                             f(slice(4 * b + 1, 4 * b + 3)))
        for b in range(BPC):
            V.tensor_add(dst[:, 4 * b + 1 : 4 * b + 3, :],
                         dst[:, 4 * b + 1 : 4 * b + 3, :],
                         um_t[:, 4 * b : 4 * b + 2, :])

    # ---- absw -> inv = exp(-ln(absw)); combine lands in `bias` scratch ----
    combine(bias, upA, u0A, umA)
    for b in range(BPC):
        ACT.activation(stages[b][:], bias[:, 4 * b : 4 * b + 4, :], AF.Ln)
        ACT.activation(inv[:, 4 * b : 4 * b + 4, :], stages[b][:], AF.Exp, scale=-1.0)

    # ---- step-1 shift-sum (inputs streamed above); lands in tmp1 center ----
    combine(_c(tmp1), upC, u0C, umC)

    # ---- gate_sum (A-set reused) -> bias = raw - gate_sum*inv*raw;
    # the u0 class reads the (guarded) gate planes directly ----
    V.tensor_add(upA[:], gv(0, 1), gv(1))
    V.tensor_add(upA[:], upA[:], gv(2, -1))
    V.tensor_add(umA[:], gv(5, 1), gv(6))
    V.tensor_add(umA[:], umA[:], gv(7, -1))

    def gvr(k, dj):
        return lambda rs: gates[:, k, rs, C0 + dj : C0 + dj + W]

    def tvr(t, dj):
        return lambda rs: t[:, rs, C0 + dj : C0 + dj + W]

    combine(_c(tmp2), upA, [gvr(3, 1), gvr(4, -1)], umA)
    V.tensor_mul(_c(tmp2), _c(tmp2), _c(rA))  # gate_sum_unnorm * raw
    V.tensor_mul(_c(tmp2), _c(tmp2), inv[:])
    V.tensor_sub(bias[:], _c(rA), _c(tmp2))

    # ---- finish step 1: r1 = inv * U1 + bias ----
    V.tensor_mul(_c(tmp1), _c(tmp1), inv[:])
    V.tensor_add(_c(rA), _c(tmp1), bias[:])

    def norm_split(acc, last):
        """r = inv*acc + bias, interior rows first (edge rows arrive late
        via PE+drain), then the stepped edge rows."""
        for b in range(BPC):
            V.tensor_mul(acc[:, 4 * b + 1 : 4 * b + 3, :],
                         acc[:, 4 * b + 1 : 4 * b + 3, :],
                         inv[:, 4 * b + 1 : 4 * b + 3, :])
            V.tensor_add(rA[:, 4 * b + 1 : 4 * b + 3, C0 : C0 + W],
                         acc[:, 4 * b + 1 : 4 * b + 3, :],
                         bias[:, 4 * b + 1 : 4 * b + 3, :])
        for q0 in (3, 0):
            V.tensor_mul(acc[:, q0::RPP, :], acc[:, q0::RPP, :], inv[:, q0::RPP, :])
            V.tensor_add(rA[:, q0::RPP, C0 : C0 + W], acc[:, q0::RPP, :],
                         bias[:, q0::RPP, :])
        if last:
            for b in range(BPC):
                st = stages[b]
                V.tensor_copy(st[:], rA[:, 4 * b : 4 * b + 4, C0 : C0 + W])
                nc.sync.dma_start(
                    out=o_dram[b, 0].rearrange("(p r) j -> p r j", p=P), in_=st[:]
                )

    def norm_split_last(acc):
        """Final step: finish image b completely, then cast+store it while
        the other image's rows are still being normalized."""
        for b in range(BPC):
            V.tensor_mul(acc[:, 4 * b + 1 : 4 * b + 3, :],
                         acc[:, 4 * b + 1 : 4 * b + 3, :],
                         inv[:, 4 * b + 1 : 4 * b + 3, :])
            V.tensor_add(rA[:, 4 * b + 1 : 4 * b + 3, C0 : C0 + W],
                         acc[:, 4 * b + 1 : 4 * b + 3, :],
                         bias[:, 4 * b + 1 : 4 * b + 3, :])
            for q in (4 * b + 3, 4 * b):
                V.tensor_mul(acc[:, q, :], acc[:, q, :], inv[:, q, :])
                V.tensor_add(rA[:, q, C0 : C0 + W], acc[:, q, :], bias[:, q, :])
            st = stages[b]
            V.tensor_copy(st[:], rA[:, 4 * b : 4 * b + 4, C0 : C0 + W])
            nc.sync.dma_start(
                out=o_dram[b, 0].rearrange("(p r) j -> p r j", p=P), in_=st[:]
            )

    # ---- steps 2..4, r updated in place in rA ----
    # Class order up, um, then the two u0 products LAST (p3 -> tmp2, p4 ->
    # tmp1, both persisting through the combine): the u0 class never
    # materializes a tree sum -- its two column-shifted product planes feed
    # the PSUM groups and interior adds directly.
    for step in range(1, PROP_TIME):
        V.tensor_mul(_c(tmp1), gv(0), _c(rA))
        V.tensor_mul(_c(tmp2), gv(1), _c(rA))
        V.tensor_add(upA[:], _w(tmp1, 1), _c(tmp2))
        V.tensor_mul(_c(tmp1), gv(2), _c(rA))
        V.tensor_add(upA[:], upA[:], _w(tmp1, -1))
        V.tensor_mul(_c(tmp1), gv(5), _c(rA))
        V.tensor_mul(_c(tmp2), gv(6), _c(rA))
        V.tensor_add(umA[:], _w(tmp1, 1), _c(tmp2))
        V.tensor_mul(_c(tmp1), gv(7), _c(rA))
        V.tensor_add(umA[:], umA[:], _w(tmp1, -1))
        V.tensor_mul(_c(tmp2), gv(3), _c(rA))
        V.tensor_mul(_c(tmp1), gv(4), _c(rA))
        combine(upC, upA, [tvr(tmp2, 1), tvr(tmp1, -1)], umA, u0_after_um=True)
        if step < PROP_TIME - 1:
            norm_split(upC, last=False)
        else:
            norm_split_last(upC)


def build(legalize=True):
    nc = bass.Bass()
    g_dram = nc.declare_dram_parameter("guidance", [BPC, K, H, W], F32, isOutput=False)
    d_dram = nc.declare_dram_parameter("blur_depth", [BPC, 1, H, W], F32, isOutput=False)
    o_dram = nc.declare_dram_parameter("out", [BPC, 1, H, W], F32, isOutput=True)
    with tile.TileContext(nc) as tc:
        with tc.tile_pool(name="main", bufs=1) as pool:
            with tc.tile_pool(name="ps", space="PSUM", bufs=1) as psum:
                _emit(nc, pool, psum, g_dram, d_dram, o_dram)
    if legalize:
        _split_excess_waits(nc)
    return nc


_NC = None


def _get_nc():
    global _NC
    if _NC is None:
        _NC = build()
    return _NC


def run(guidance, blur_depth, **spmd_kwargs):
    nc = _get_nc()
    in_maps = [
        {
            "guidance": np.ascontiguousarray(guidance[BPC * c : BPC * (c + 1)]),
            "blur_depth": np.ascontiguousarray(blur_depth[BPC * c : BPC * (c + 1)]),
        }
        for c in range(N_CORES)
    ]
    res = run_bass_kernel_spmd(nc, in_maps, list(range(N_CORES)), **spmd_kwargs)
    out = np.concatenate([res.results[i]["out"] for i in range(N_CORES)], axis=0)
    return out, res


def kernel(guidance, blur_depth):
    out, _ = run(guidance, blur_depth)
    return out.astype(np.float32)
